# revision 1
# baseline (speedup 1.0000x reference)
"""GraphSAGE (3-layer) Trainium2 Bass kernel, 8-core SPMD.

Strategy (graph/data parallel, per sharding hint):
  - Nodes padded to 50176 = 8*6272; core c owns dst nodes [c*6272, (c+1)*6272).
  - Edges bucketed by (dst tile of 128 nodes); mean-aggregation done as PE
    matmuls: for each chunk of 128 edges, psum += onehotT.T @ msgs, where
    msgs = dma_gather(h_table[src]) and onehotT built on the Vector engine as
    is_equal(iota_row_matrix, dstloc_per_edge) (pad slots hold 128 -> zero
    column -> no contribution). Gathers are SWDGE-descriptor-rate bound, so
    keeping the one-hot off the gather path halves GPSIMD time.
  - dma_gather indices are int16 (<=32767), so each tile's edges are split in
    a "lo" group (src < 25088, gathered from table base 0) and a "hi" group
    (src >= 25088, gathered from table base 17408 with idx = src - 17408).
  - Linear: out^T = W_self^T @ h_self^T + W_neigh^T @ h_neigh^T on PE
    (transposes via PE identity matmul), bias+ReLU on ACT, then transpose
    back to row-major for the next layer's gather table.
  - Inter-layer: each core's block is AllGather'ed (HBM collective) into a
    full replicated bf16 table for the next layer's gathers.
"""

import sys

if "/opt/trn_rl_repo" not in sys.path:
    sys.path.insert(0, "/opt/trn_rl_repo")

from contextlib import ExitStack

import numpy as np
import ml_dtypes

N_NODES = 50000
F = 128
OUT_F = 64
NCORES = 8
NLOC = 6272          # nodes per core
NTILES = 49          # 6272 / 128
NPAD = NCORES * NLOC  # 50176
P = 128
SPLIT = 25088        # lo: src < SPLIT ; hi: src >= SPLIT
HI_BASE = 17408      # hi gather base; idx = src - HI_BASE  (max 50175-17408=32767)
IDROWS = 192         # identity gather table rows (128 identity + zero rows)

_prog_cache = {}


def _wrap_idx(a):
    """[T, n] idx stream -> dma_gather wrapped layout [128, T, n/16] int16.

    wrapped[p, t, s] = a[t, s*16 + p%16]  (replicated across the 8 Q7 cores).
    """
    T, n = a.shape
    w = a.reshape(T, n // 16, 16).transpose(2, 0, 1)      # [16, T, n/16]
    w = np.tile(w, (8, 1, 1))                              # [128, T, n/16]
    return np.ascontiguousarray(w.astype(np.int16))


def _preprocess(src, dst):
    """Bucket edges by (core,tile), split lo/hi by src, pad to uniform chunks."""
    src = src.astype(np.int64)
    dst = dst.astype(np.int64)
    E = src.shape[0]

    gtile = dst // P            # global tile id 0..391 (dst tile of 128 nodes)
    dstloc = dst % P
    lo = src < SPLIT

    key = gtile * 2 + (~lo).astype(np.int64)   # lo group first within tile
    order = np.argsort(key, kind="stable")
    counts = np.bincount(key, minlength=NCORES * NTILES * 2)
    lo_counts = counts[0::2].reshape(NCORES, NTILES)
    hi_counts = counts[1::2].reshape(NCORES, NTILES)

    NLO = int(np.ceil(lo_counts.max() / P))
    NHI = int(np.ceil(hi_counts.max() / P))
    NCH = NLO + NHI

    # slot arrays per global tile
    src_slot = np.zeros((NCORES * NTILES, NCH * P), np.int64)
    oh_slot = np.full((NCORES * NTILES, NCH * P), P, np.int64)  # 128 -> zero row

    skey = key[order]
    group_start = np.zeros(NCORES * NTILES * 2 + 1, np.int64)
    np.cumsum(counts, out=group_start[1:])
    pos_in_group = np.arange(E) - group_start[skey]
    row = gtile[order]
    grp = skey % 2
    col = pos_in_group + grp * (NLO * P)
    sv = src[order]
    src_slot[row, col] = np.where(grp == 0, sv, sv - HI_BASE)
    oh_slot[row, col] = dstloc[order]

    deg = np.bincount(dst, minlength=NPAD).astype(np.float32)
    inv_deg = 1.0 / np.maximum(deg, 1.0)

    per_core = []
    for c in range(NCORES):
        sl = src_slot[c * NTILES:(c + 1) * NTILES]
        ol = oh_slot[c * NTILES:(c + 1) * NTILES]
        idxlo = _wrap_idx(sl[:, : NLO * P])
        idxhi = _wrap_idx(sl[:, NLO * P:])
        # dstloc values, edge-partitioned: [128, NTILES, NCH] bf16
        # (pad slots hold 128 -> never equal to iota 0..127 -> zero column)
        import ml_dtypes as _md
        dstlocf = np.ascontiguousarray(
            ol.reshape(NTILES, NCH_ := ol.shape[1] // P, P).transpose(2, 0, 1)
        ).astype(np.float32)
        invd = inv_deg[c * NLOC:(c + 1) * NLOC].reshape(NTILES, P).T.copy()  # [128, 49]
        per_core.append(dict(idxlo=idxlo, idxhi=idxhi, dstlocf=dstlocf, invdeg=invd))
    return per_core, NLO, NHI


def _build_program(NLO, NHI):
    import concourse.bacc as bacc
    import concourse.bass as bass
    import concourse.mybir as mybir
    import concourse.tile as tile

    dt = mybir.dt
    NCH = NLO + NHI
    nc = bacc.Bacc("TRN2", target_bir_lowering=False, debug=False,
                   num_devices=NCORES, dynamic_dma_scratch_size=49152)

    htab0 = nc.dram_tensor("htab0", [NPAD, F], dt.bfloat16, kind="ExternalInput")
    hself0 = nc.dram_tensor("hself0", [P, NTILES, F], dt.bfloat16, kind="ExternalInput")
    idxlo = nc.dram_tensor("idxlo", [P, NTILES, NLO * 8], dt.int16, kind="ExternalInput")
    idxhi = nc.dram_tensor("idxhi", [P, NTILES, NHI * 8], dt.int16, kind="ExternalInput")
    dstlocf = nc.dram_tensor("dstlocf", [P, NTILES, NCH], dt.float32, kind="ExternalInput")
    iotam = nc.dram_tensor("iotam", [P, P], dt.float32, kind="ExternalInput")
    invdeg = nc.dram_tensor("invdeg", [P, NTILES], dt.float32, kind="ExternalInput")
    ident = nc.dram_tensor("ident", [P, P], dt.bfloat16, kind="ExternalInput")
    identf = nc.dram_tensor("identf", [OUT_F, OUT_F], dt.float32, kind="ExternalInput")
    ws = [nc.dram_tensor(f"ws{l}", [F, F if l < 2 else OUT_F], dt.bfloat16,
                         kind="ExternalInput") for l in range(3)]
    wn = [nc.dram_tensor(f"wn{l}", [F, F if l < 2 else OUT_F], dt.bfloat16,
                         kind="ExternalInput") for l in range(3)]
    bs = [nc.dram_tensor(f"b{l}", [F if l < 2 else OUT_F, 1], dt.float32,
                         kind="ExternalInput") for l in range(3)]
    outd = nc.dram_tensor("out", [NLOC, OUT_F], dt.float32, kind="ExternalOutput")

    htabs = [htab0,
             nc.dram_tensor("htab1", [NPAD, F], dt.bfloat16, addr_space="Shared"),
             nc.dram_tensor("htab2", [NPAD, F], dt.bfloat16, addr_space="Shared")]
    blks = [nc.dram_tensor(f"blk{l}", [NLOC, F], dt.bfloat16) for l in range(2)]

    with tile.TileContext(nc) as tc, ExitStack() as ctx:
        const = ctx.enter_context(tc.tile_pool(name="const", bufs=1))
        stpool = ctx.enter_context(tc.tile_pool(name="stage", bufs=1))
        msgp = ctx.enter_context(tc.tile_pool(name="msg", bufs=4))
        ohp = ctx.enter_context(tc.tile_pool(name="oh", bufs=2))
        sbw = ctx.enter_context(tc.tile_pool(name="work", bufs=3))
        psA = ctx.enter_context(tc.tile_pool(name="psA", bufs=2, space="PSUM"))
        psT = ctx.enter_context(tc.tile_pool(name="psT", bufs=1, space="PSUM"))
        psO = ctx.enter_context(tc.tile_pool(name="psO", bufs=2, space="PSUM"))

        def load(t, d):
            nc.sync.dma_start(t[:], d[:])
            return t

        idxlo_sb = load(const.tile([P, NTILES, NLO * 8], dt.int16, name="idxlo_sb"), idxlo)
        idxhi_sb = load(const.tile([P, NTILES, NHI * 8], dt.int16, name="idxhi_sb"), idxhi)
        dstlocf_sb = load(const.tile([P, NTILES, NCH], dt.float32, name="dstlocf_sb"), dstlocf)
        iotam_sb = load(const.tile([P, P], dt.float32, name="iotam_sb"), iotam)
        invdeg_sb = load(const.tile([P, NTILES], dt.float32, name="invdeg_sb"), invdeg)
        ident_sb = load(const.tile([P, P], dt.bfloat16, name="ident_sb"), ident)
        identf_sb = load(const.tile([OUT_F, OUT_F], dt.float32, name="identf_sb"), identf)
        ws_sb = [load(const.tile([F, F if l < 2 else OUT_F], dt.bfloat16, name=f"ws_sb{l}"), ws[l])
                 for l in range(3)]
        wn_sb = [load(const.tile([F, F if l < 2 else OUT_F], dt.bfloat16, name=f"wn_sb{l}"), wn[l])
                 for l in range(3)]
        bs_sb = [load(const.tile([F if l < 2 else OUT_F, 1], dt.float32, name=f"bs_sb{l}"), bs[l])
                 for l in range(3)]

        stageA = load(stpool.tile([P, NTILES, F], dt.bfloat16, name="stageA", tag="stA"), hself0)
        stageB = stpool.tile([P, NTILES, F], dt.bfloat16, tag="stB")
        outstage = stpool.tile([P, NTILES, OUT_F], dt.float32, tag="stO")

        stage_prev, stage_next = stageA, stageB
        for l in range(3):
            tab = htabs[l]
            OUTL = F if l < 2 else OUT_F
            for t in range(NTILES):
                msg = msgp.tile([P, NCH, F], dt.bfloat16, tag="msg")
                oh = ohp.tile([P, NCH, F], dt.bfloat16, tag="oh")
                GC = 23  # chunks per gather call (<=3071 idxs, scratch cap)
                for j in range(0, NLO, GC):
                    n = min(GC, NLO - j)
                    nc.gpsimd.dma_gather(
                        msg[:, j:j + n, :], tab[0:32768, :],
                        idxlo_sb[:, t, j * 8:(j + n) * 8],
                        num_idxs=n * P, num_idxs_reg=n * P, elem_size=F,
                        single_packet=False)
                for j in range(0, NHI, GC):
                    n = min(GC, NHI - j)
                    nc.gpsimd.dma_gather(
                        msg[:, NLO + j:NLO + j + n, :],
                        tab[HI_BASE:HI_BASE + 32768, :],
                        idxhi_sb[:, t, j * 8:(j + n) * 8],
                        num_idxs=n * P, num_idxs_reg=n * P, elem_size=F,
                        single_packet=False)
                for k in range(NCH):
                    nc.vector.tensor_scalar(
                        oh[:, k, :], iotam_sb[:],
                        dstlocf_sb[:, t, k:k + 1], None,
                        mybir.AluOpType.is_equal)

                agg = psA.tile([P, F], dt.float32, tag="agg")
                for k in range(NCH):
                    nc.tensor.matmul(agg[:], oh[:, k, :], msg[:, k, :],
                                     start=(k == 0), stop=(k == NCH - 1))
                hn = sbw.tile([P, F], dt.bfloat16, tag="hn")
                nc.vector.tensor_scalar_mul(hn[:], agg[:], invdeg_sb[:, t:t + 1])

                hsT_ps = psT.tile([F, P], dt.bfloat16, tag="hsT")
                nc.tensor.transpose(hsT_ps[:], stage_prev[:, t, :], ident_sb[:])
                hsT = sbw.tile([F, P], dt.bfloat16, tag="hsTs")
                nc.vector.tensor_copy(hsT[:], hsT_ps[:])

                hnT_ps = psT.tile([F, P], dt.bfloat16, tag="hnT")
                nc.tensor.transpose(hnT_ps[:], hn[:], ident_sb[:])
                hnT = sbw.tile([F, P], dt.bfloat16, tag="hnTs")
                nc.vector.tensor_copy(hnT[:], hnT_ps[:])

                outp = psO.tile([OUTL, P], dt.float32, tag="outp")
                nc.tensor.matmul(outp[:], ws_sb[l][:], hsT[:], start=True, stop=False)
                nc.tensor.matmul(outp[:], wn_sb[l][:], hnT[:], start=False, stop=True)

                if l < 2:
                    outT = sbw.tile([OUTL, P], dt.bfloat16, tag="outT")
                    nc.scalar.activation(outT[:], outp[:],
                                         mybir.ActivationFunctionType.Relu,
                                         bias=bs_sb[l][:], scale=1.0)
                    oT_ps = psT.tile([P, OUTL], dt.bfloat16, tag="oT")
                    nc.tensor.transpose(oT_ps[:], outT[:], ident_sb[:])
                    nc.vector.tensor_copy(stage_next[:, t, :], oT_ps[:])
                else:
                    outT = sbw.tile([OUTL, P], dt.float32, tag="outTf")
                    nc.vector.tensor_scalar_add(outT[:], outp[:], bs_sb[2][:])
                    oT_ps = psT.tile([P, OUTL], dt.float32, tag="oT2")
                    nc.tensor.transpose(oT_ps[:], outT[:], identf_sb[:])
                    nc.vector.tensor_copy(outstage[:, t, :], oT_ps[:])

            if l < 2:
                blk = blks[l]
                nc.sync.dma_start(
                    blk[:].rearrange("(t p) f -> p t f", p=P), stage_next[:])
                nc.gpsimd.collective_compute(
                    "AllGather", mybir.AluOpType.bypass,
                    replica_groups=[list(range(NCORES))],
                    ins=[blk[:]], outs=[htabs[l + 1][:]])
                stage_prev, stage_next = stage_next, stage_prev

        nc.sync.dma_start(outd[:].rearrange("(t p) f -> p t f", p=P), outstage[:])

    nc.compile()
    return nc


def kernel(features, src, dst, W0, b0, W1, b1, W2, b2):
    features = np.asarray(features, np.float32)
    src = np.asarray(src)
    dst = np.asarray(dst)

    per_core, NLO, NHI = _preprocess(src, dst)

    key = (NLO, NHI)
    if key not in _prog_cache:
        _prog_cache[key] = _build_program(NLO, NHI)
    nc = _prog_cache[key]

    bf = ml_dtypes.bfloat16
    feat_pad = np.zeros((NPAD, F), np.float32)
    feat_pad[:N_NODES] = features
    htab0 = feat_pad.astype(bf)
    ident = np.eye(P, dtype=bf)
    iotam = np.tile(np.arange(P, dtype=np.float32), (P, 1))
    Wl = [np.asarray(w, np.float32) for w in (W0, W1, W2)]
    bl = [np.asarray(b, np.float32).reshape(-1, 1) for b in (b0, b1, b2)]

    identf = np.eye(OUT_F, dtype=np.float32)
    common = dict(htab0=htab0, iotam=iotam, ident=ident, identf=identf)
    for l in range(3):
        common[f"ws{l}"] = Wl[l][:F].astype(bf)
        common[f"wn{l}"] = Wl[l][F:].astype(bf)
        common[f"b{l}"] = bl[l]

    in_maps = []
    for c in range(NCORES):
        m = dict(common)
        m.update(per_core[c])
        hs = feat_pad[c * NLOC:(c + 1) * NLOC].reshape(NTILES, P, F)
        m["hself0"] = np.ascontiguousarray(hs.transpose(1, 0, 2)).astype(bf)
        in_maps.append(m)

    from concourse.bass_utils import run_bass_kernel_spmd
    res = run_bass_kernel_spmd(nc, in_maps, core_ids=list(range(NCORES)))
    global last_result
    last_result = res
    out = np.concatenate([res.results[c]["out"] for c in range(NCORES)], axis=0)
    return np.ascontiguousarray(out[:N_NODES]).astype(np.float32)


last_result = None



# revision 3
# speedup vs baseline: 1.1034x; 1.1034x over previous
"""GraphSAGE (3-layer) Trainium2 Bass kernel, 8-core SPMD. v2

Strategy (graph/data parallel):
  - Nodes padded to 50176 = 8*6272; core c owns dst nodes [c*6272, (c+1)*6272),
    49 dst tiles of 128 nodes per core.
  - Mean-aggregation per dst tile as PE matmuls: psum += oh_k.T @ msg_k over
    chunks k of 128 edges, where msg = dma_gather(h_table[src]) and oh is the
    per-chunk one-hot of dst slots.
  - One-hot built in ONE DVE op per tile (not per chunk):
    oh[p, k, s] = is_equal(iota[s], dstloc[p, k]) with stride-0 broadcast APs.
    Pad slots carry dstloc=128 -> zero one-hot row.
  - Gather idx streams pad with trailing -1: the Q7 desc-gen kernel trims
    trailing negatives, so padding costs no GPSIMD time. msg pool buffers are
    memset once so untouched pad slots stay finite (0 * 0 = 0 in the matmul).
  - Activations flow FEATURE-major (stageT [f, tile, node]): linear layers run
    directly (lhsT=W[in_f, out_f], rhs=stageT), ReLU+bias on ACT writes the
    next stage in place. Only 2 PE transposes per tile (hn, table-row write).
  - The h table is split in two (A = tiles 0..23 per core, B = tiles 24..48).
    AllGather A is issued mid-layer (after tile 27) so it overlaps the back
    half of the tile loop; next layer's lo-gathers (table A sources) depend
    only on it, hi-gathers on AllGather B issued at layer end.
"""

import sys

if "/opt/trn_rl_repo" not in sys.path:
    sys.path.insert(0, "/opt/trn_rl_repo")

from contextlib import ExitStack

import numpy as np
import ml_dtypes

N_NODES = 50000
F = 128
OUT_F = 64
NCORES = 8
NLOC = 6272          # nodes per core
NTILES = 49          # 6272 / 128
NPAD = NCORES * NLOC  # 50176
P = 128
AT = 24              # tiles per core in table A
BT = NTILES - AT     # 25 tiles in table B
RA = AT * P          # 3072 rows per core in A
RB = BT * P          # 3200 rows per core in B
NA = NCORES * RA     # 24576
NB = NCORES * RB     # 25600
COLL_A_AFTER = 27    # emit AllGather-A after this tile's gathers are queued

_prog_cache = {}


def _wrap_idx(a):
    """[T, n] idx stream -> dma_gather wrapped layout [128, T, n/16] int16.

    wrapped[p, t, s] = a[t, s*16 + p%16]  (replicated across the 8 Q7 cores).
    """
    T, n = a.shape
    w = a.reshape(T, n // 16, 16).transpose(2, 0, 1)      # [16, T, n/16]
    w = np.tile(w, (8, 1, 1))                              # [128, T, n/16]
    return np.ascontiguousarray(w.astype(np.int16))


def _preprocess(src, dst):
    """Bucket edges by (core,tile), split A/B by src row, pad with -1."""
    bf = ml_dtypes.bfloat16
    src = src.astype(np.int64)
    dst = dst.astype(np.int64)
    E = src.shape[0]

    gtile = dst // P            # global dst tile id 0..391
    dstloc = dst % P
    c_src = src // NLOC
    r_src = src % NLOC
    lo = r_src < RA             # table A sources
    idx_a = c_src * RA + r_src          # valid when lo
    idx_b = c_src * RB + (r_src - RA)   # valid when ~lo

    key = gtile * 2 + (~lo).astype(np.int64)   # lo group first within tile
    order = np.argsort(key, kind="stable")
    counts = np.bincount(key, minlength=NCORES * NTILES * 2)
    lo_counts = counts[0::2].reshape(NCORES, NTILES)
    hi_counts = counts[1::2].reshape(NCORES, NTILES)

    NLO = int(np.ceil(lo_counts.max() / P))
    NHI = int(np.ceil(hi_counts.max() / P))
    NCH = NLO + NHI

    src_slot = np.zeros((NCORES * NTILES, NCH * P), np.int64)   # pad: gather row 0
    oh_slot = np.full((NCORES * NTILES, NCH * P), P, np.int64)  # 128 -> zero row

    skey = key[order]
    group_start = np.zeros(NCORES * NTILES * 2 + 1, np.int64)
    np.cumsum(counts, out=group_start[1:])
    pos_in_group = np.arange(E) - group_start[skey]
    row = gtile[order]
    grp = skey % 2
    col = pos_in_group + grp * (NLO * P)
    sv = np.where(grp == 0, idx_a[order], idx_b[order])
    src_slot[row, col] = sv
    oh_slot[row, col] = dstloc[order]

    deg = np.bincount(dst, minlength=NPAD).astype(np.float32)
    inv_deg = 1.0 / np.maximum(deg, 1.0)

    per_core = []
    for c in range(NCORES):
        sl = src_slot[c * NTILES:(c + 1) * NTILES]
        ol = oh_slot[c * NTILES:(c + 1) * NTILES]
        idxlo = _wrap_idx(sl[:, : NLO * P])
        idxhi = _wrap_idx(sl[:, NLO * P:])
        # dstloc values, edge-partitioned: [128, NTILES, NCH] bf16
        dstlocb = np.ascontiguousarray(
            ol.reshape(NTILES, NCH, P).transpose(2, 0, 1)
        ).astype(bf)
        invd = inv_deg[c * NLOC:(c + 1) * NLOC].reshape(NTILES, P).T.copy()  # [128, 49]
        per_core.append(dict(idxlo=idxlo, idxhi=idxhi, dstlocb=dstlocb, invdeg=invd))
    return per_core, NLO, NHI


def _build_program(NLO, NHI):
    import concourse.bacc as bacc
    import concourse.mybir as mybir
    import concourse.tile as tile

    dt = mybir.dt
    NCH = NLO + NHI
    nc = bacc.Bacc("TRN2", target_bir_lowering=False, debug=False,
                   num_devices=NCORES, dynamic_dma_scratch_size=49152)

    htabA0 = nc.dram_tensor("htabA0", [NA, F], dt.bfloat16, kind="ExternalInput")
    htabB0 = nc.dram_tensor("htabB0", [NB, F], dt.bfloat16, kind="ExternalInput")
    hselfT0 = nc.dram_tensor("hselfT0", [F, NTILES, P], dt.bfloat16, kind="ExternalInput")
    idxlo = nc.dram_tensor("idxlo", [P, NTILES, NLO * 8], dt.int16, kind="ExternalInput")
    idxhi = nc.dram_tensor("idxhi", [P, NTILES, NHI * 8], dt.int16, kind="ExternalInput")
    dstlocb = nc.dram_tensor("dstlocb", [P, NTILES, NCH], dt.bfloat16, kind="ExternalInput")
    iotam = nc.dram_tensor("iotam", [P, P], dt.bfloat16, kind="ExternalInput")
    invdeg = nc.dram_tensor("invdeg", [P, NTILES], dt.float32, kind="ExternalInput")
    ident = nc.dram_tensor("ident", [P, P], dt.bfloat16, kind="ExternalInput")
    ws = [nc.dram_tensor(f"ws{l}", [F, F if l < 2 else OUT_F], dt.bfloat16,
                         kind="ExternalInput") for l in range(3)]
    wn = [nc.dram_tensor(f"wn{l}", [F, F if l < 2 else OUT_F], dt.bfloat16,
                         kind="ExternalInput") for l in range(3)]
    bs = [nc.dram_tensor(f"b{l}", [F if l < 2 else OUT_F, 1], dt.float32,
                         kind="ExternalInput") for l in range(3)]
    outd = nc.dram_tensor("outT", [OUT_F, NTILES, P], dt.float32, kind="ExternalOutput")

    tabsA = [htabA0,
             nc.dram_tensor("htabA1", [NA, F], dt.bfloat16, addr_space="Shared"),
             nc.dram_tensor("htabA2", [NA, F], dt.bfloat16, addr_space="Shared")]
    tabsB = [htabB0,
             nc.dram_tensor("htabB1", [NB, F], dt.bfloat16, addr_space="Shared"),
             nc.dram_tensor("htabB2", [NB, F], dt.bfloat16, addr_space="Shared")]
    blkA = [nc.dram_tensor(f"blkA{l}", [RA, F], dt.bfloat16) for l in range(2)]
    blkB = [nc.dram_tensor(f"blkB{l}", [RB, F], dt.bfloat16) for l in range(2)]

    with tile.TileContext(nc) as tc, ExitStack() as ctx:
        const = ctx.enter_context(tc.tile_pool(name="const", bufs=1))
        stpool = ctx.enter_context(tc.tile_pool(name="stage", bufs=1))
        msgp = ctx.enter_context(tc.tile_pool(name="msg", bufs=6))
        ohp = ctx.enter_context(tc.tile_pool(name="oh", bufs=3))
        sbw = ctx.enter_context(tc.tile_pool(name="work", bufs=4))
        psA = ctx.enter_context(tc.tile_pool(name="psA", bufs=2, space="PSUM"))
        psT = ctx.enter_context(tc.tile_pool(name="psT", bufs=2, space="PSUM"))
        psO = ctx.enter_context(tc.tile_pool(name="psO", bufs=2, space="PSUM"))

        def load(t, d):
            nc.sync.dma_start(t[:], d[:])
            return t

        idxlo_sb = load(const.tile([P, NTILES, NLO * 8], dt.int16, name="idxlo_sb"), idxlo)
        idxhi_sb = load(const.tile([P, NTILES, NHI * 8], dt.int16, name="idxhi_sb"), idxhi)
        dstlocb_sb = load(const.tile([P, NTILES, NCH], dt.bfloat16, name="dstlocb_sb"), dstlocb)
        iotam_sb = load(const.tile([P, P], dt.bfloat16, name="iotam_sb"), iotam)
        invdeg_sb = load(const.tile([P, NTILES], dt.float32, name="invdeg_sb"), invdeg)
        ident_sb = load(const.tile([P, P], dt.bfloat16, name="ident_sb"), ident)
        ws_sb = [load(const.tile([F, F if l < 2 else OUT_F], dt.bfloat16, name=f"ws_sb{l}"), ws[l])
                 for l in range(3)]
        wn_sb = [load(const.tile([F, F if l < 2 else OUT_F], dt.bfloat16, name=f"wn_sb{l}"), wn[l])
                 for l in range(3)]
        bs_sb = [load(const.tile([F if l < 2 else OUT_F, 1], dt.float32, name=f"bs_sb{l}"), bs[l])
                 for l in range(3)]

        stageA = load(stpool.tile([F, NTILES, P], dt.bfloat16, name="stageA", tag="stA"), hselfT0)
        stageB = stpool.tile([F, NTILES, P], dt.bfloat16, tag="stB")
        outstage = stpool.tile([OUT_F, NTILES, P], dt.float32, tag="stO")

        # prime the msg pool buffers with zeros: pad slots are never gathered
        # (trailing -1 idxs are trimmed) and must stay finite for 0*x = 0.
        for _ in range(6):
            m = msgp.tile([P, NCH, F], dt.bfloat16, tag="msg")
            nc.vector.memset(m[:], 0.0)

        stage_prev, stage_next = stageA, stageB
        for l in range(3):
            tabA, tabB = tabsA[l], tabsB[l]
            OUTL = F if l < 2 else OUT_F
            for t in range(NTILES):
                msg = msgp.tile([P, NCH, F], dt.bfloat16, tag="msg")
                nc.gpsimd.dma_gather(
                    msg[:, 0:NLO, :], tabA[:, :], idxlo_sb[:, t, :],
                    num_idxs=NLO * P, num_idxs_reg=NLO * P, elem_size=F,
                    single_packet=False)
                nc.gpsimd.dma_gather(
                    msg[:, NLO:NCH, :], tabB[:, :], idxhi_sb[:, t, :],
                    num_idxs=NHI * P, num_idxs_reg=NHI * P, elem_size=F,
                    single_packet=False)

                if l < 2 and t == COLL_A_AFTER:
                    # blkA rows (tiles 0..23) are long written by now; the
                    # collective overlaps the remaining tiles of this layer.
                    nc.gpsimd.collective_compute(
                        "AllGather", mybir.AluOpType.bypass,
                        replica_groups=[list(range(NCORES))],
                        ins=[blkA[l][:]], outs=[tabsA[l + 1][:]])

                oh = ohp.tile([P, NCH, P], dt.bfloat16, tag="oh")
                nc.vector.tensor_tensor(
                    oh[:],
                    iotam_sb[:].unsqueeze(1).to_broadcast([P, NCH, P]),
                    dstlocb_sb[:, t, :].unsqueeze(2).to_broadcast([P, NCH, P]),
                    mybir.AluOpType.is_equal)

                agg = psA.tile([P, F], dt.float32, tag="agg")
                for k in range(NCH):
                    nc.tensor.matmul(agg[:], oh[:, k, :], msg[:, k, :],
                                     start=(k == 0), stop=(k == NCH - 1))
                hn = sbw.tile([P, F], dt.bfloat16, tag="hn")
                nc.vector.tensor_scalar_mul(hn[:], agg[:], invdeg_sb[:, t:t + 1])

                hnT_ps = psT.tile([F, P], dt.bfloat16, tag="hnT")
                nc.tensor.transpose(hnT_ps[:], hn[:], ident_sb[:])
                hnT = sbw.tile([F, P], dt.bfloat16, tag="hnTs")
                nc.vector.tensor_copy(hnT[:], hnT_ps[:])

                outp = psO.tile([OUTL, P], dt.float32, tag="outp")
                nc.tensor.matmul(outp[:], ws_sb[l][:], stage_prev[:, t, :],
                                 start=True, stop=False)
                nc.tensor.matmul(outp[:], wn_sb[l][:], hnT[:],
                                 start=False, stop=True)

                if l < 2:
                    nc.scalar.activation(stage_next[:, t, :], outp[:],
                                         mybir.ActivationFunctionType.Relu,
                                         bias=bs_sb[l][:], scale=1.0)
                    # table rows for the next layer (node-major)
                    oT_ps = psT.tile([P, F], dt.bfloat16, tag="oT")
                    nc.tensor.transpose(oT_ps[:], stage_next[:, t, :], ident_sb[:])
                    rowm = sbw.tile([P, F], dt.bfloat16, tag="rowm")
                    nc.vector.tensor_copy(rowm[:], oT_ps[:])
                    if t < AT:
                        nc.sync.dma_start(blkA[l][t * P:(t + 1) * P, :], rowm[:])
                    else:
                        nc.sync.dma_start(
                            blkB[l][(t - AT) * P:(t - AT + 1) * P, :], rowm[:])
                else:
                    nc.vector.tensor_scalar_add(outstage[:, t, :], outp[:],
                                                bs_sb[2][:])

            if l < 2:
                nc.gpsimd.collective_compute(
                    "AllGather", mybir.AluOpType.bypass,
                    replica_groups=[list(range(NCORES))],
                    ins=[blkB[l][:]], outs=[tabsB[l + 1][:]])
                stage_prev, stage_next = stage_next, stage_prev

        nc.sync.dma_start(outd[:], outstage[:])

    nc.compile()
    return nc


def kernel(features, src, dst, W0, b0, W1, b1, W2, b2):
    features = np.asarray(features, np.float32)
    src = np.asarray(src)
    dst = np.asarray(dst)

    per_core, NLO, NHI = _preprocess(src, dst)

    key = (NLO, NHI)
    if key not in _prog_cache:
        _prog_cache[key] = _build_program(NLO, NHI)
    nc = _prog_cache[key]

    bf = ml_dtypes.bfloat16
    feat_pad = np.zeros((NPAD, F), np.float32)
    feat_pad[:N_NODES] = features
    fp = feat_pad.reshape(NCORES, NLOC, F)
    htabA0 = np.ascontiguousarray(fp[:, :RA].reshape(NA, F)).astype(bf)
    htabB0 = np.ascontiguousarray(fp[:, RA:].reshape(NB, F)).astype(bf)
    ident = np.eye(P, dtype=bf)
    iotam = np.tile(np.arange(P, dtype=np.float32), (P, 1)).astype(bf)
    Wl = [np.asarray(w, np.float32) for w in (W0, W1, W2)]
    bl = [np.asarray(b, np.float32).reshape(-1, 1) for b in (b0, b1, b2)]

    common = dict(htabA0=htabA0, htabB0=htabB0, iotam=iotam, ident=ident)
    for l in range(3):
        common[f"ws{l}"] = Wl[l][:F].astype(bf)
        common[f"wn{l}"] = Wl[l][F:].astype(bf)
        common[f"b{l}"] = bl[l]

    in_maps = []
    for c in range(NCORES):
        m = dict(common)
        m.update(per_core[c])
        hs = feat_pad[c * NLOC:(c + 1) * NLOC].reshape(NTILES, P, F)
        m["hselfT0"] = np.ascontiguousarray(hs.transpose(2, 0, 1)).astype(bf)
        in_maps.append(m)

    from concourse.bass_utils import run_bass_kernel_spmd
    res = run_bass_kernel_spmd(nc, in_maps, core_ids=list(range(NCORES)))
    global last_result
    last_result = res
    out = np.empty((NPAD, OUT_F), np.float32)
    for c in range(NCORES):
        oT = res.results[c]["outT"]  # [OUT_F, NTILES, P]
        out[c * NLOC:(c + 1) * NLOC] = np.asarray(oT).transpose(1, 2, 0).reshape(NLOC, OUT_F)
    return np.ascontiguousarray(out[:N_NODES]).astype(np.float32)


last_result = None
